# revision 29
# baseline (speedup 1.0000x reference)
"""AdderNet 2D conv (L1-distance "convolution") on 8 TRN2 NeuronCores.

Reference computation:
    X_col = unfold(x, k=3, stride=1, pad=1)      # (N, D, P)  D=576, P=196
    out[n, f, p] = -sum_d |W_col[f, d] - X_col[n, d, p]|

Distribution: filter-parallel — core i computes filters f in [8i, 8i+8)
for the FULL batch (no collectives; host concatenates filter slices).
This makes the per-instruction free dim N*P = 3136, which amortizes
per-instruction overhead far better than batch-parallel (392).

Per-core algorithm (raw Bass; this walrus encodes only ONE inline
sync-wait per instruction, so Tile's auto-semaphores don't compile —
standalone wait_ge instructions are used instead):

  -sum_d |x-w|  =  -sum_d x  + sum_d w  + 2*sum_d min(x-w, 0)

  - Host im2col: d (patch dim, 576, (kh,kw,c)-ordered) on SBUF
    partitions, 5 zero-padded chunks of 128, shipped as dense
    (128, 3136) bf16 tiles (window DMAs would shatter into 28-byte
    descriptors and starve everything).
  - Per (filter, chunk) unit, ONE elementwise instruction:
      VectorE: tensor_scalar(op0=sub W[f,.], op1=min 0)  -> min(x-w, 0)
      ScalarE: activation(Relu, scale=-1, bias=W[f,.])   -> relu(w-x)
    (no encodable fused abs exists on the Vector engine in this ISA;
    the f's are split between the engines to balance them).
  - TensorE reduces over partitions into PSUM, 4-way column-tiled:
    filter pair (2c, 2c+1) lives on PSUM partitions {32c, 32c+1} and
    its matmuls run on array column-group c — four concurrent moving
    streams (tile_position=(0, 32c)).  Stationary column = +2 (vector
    units) / -2 (scalar units); an all(-1) 2-col stationary block
    accumulates -sum_d x once per (chunk, stream).  PSUM is one
    (128, 7*512) tensor: matmul moving slices of 512 columns map
    exactly onto banks.  A zero-fill prologue (start=True, zero
    moving) initializes every bank's has_written bits so all real
    matmuls can accumulate without ordering hazards across streams.
  - Evacuation: per stream, psum rows {32c, 32c+1} + bias sum_d W[f]
    (activation bias / tensor_scalar add) -> osb rows {32c, 32c+1};
    streams 0-1 on ScalarE, 2-3 on VectorE.  The out DMA gathers the
    8 scattered osb rows with a 2-level partition access pattern.

kernel(x, W) accepts the FULL inputs and returns the FULL output.
"""

import os

import numpy as np
import ml_dtypes

import concourse.bass as bass
from concourse import mybir
from concourse.bass_utils import run_bass_kernel_spmd

# Problem constants (hardcoded per harness rules)
N, C, H, W_SP = 16, 64, 14, 14
F = 64
KK = 3
PAD = 1
P = H * W_SP            # 196 output positions per image
POS = N * P             # 3136 total positions
D = C * KK * KK         # 576
N_CORES = 8
F_PER = F // N_CORES    # 8 filters per core
NCHUNK = 5              # ceil(576 / 128) d-chunks
NSTREAM = 4             # TensorE column-tiling streams
QPOS = POS // NSTREAM   # 784 positions per stream (its private quarter)
BANKC = 512             # psum bank capacity in f32
SUB = [(0, 512), (512, 272)]  # per-stream sub-slices (2 private banks)
RING_V = 8              # vector-produced tile ring
RING_S = 3              # scalar-produced tile ring

FP32 = mybir.dt.float32
BF16 = mybir.dt.bfloat16

# Filters handled by the Scalar engine (the rest go to VectorE).
ACT_F = tuple(
    int(t) for t in os.environ.get("ADDER_ACT_F", "3,7").split(",") if t != ""
)

WCOLS = NCHUNK * F_PER  # 40 W columns (col = k*8+j)


def build_bass():
    nc = bass.Bass()

    x_ext = nc.declare_dram_parameter("xcol", [NCHUNK, 128, POS], BF16,
                                      isOutput=False)
    # cols 0:40 = W columns (col = k*8 + j); col 40 row (32*(j//2)+j%2) =
    # sum_d W[f_j]
    w_ext = nc.declare_dram_parameter("wcols", [128, WCOLS + 1], FP32,
                                      isOutput=False)
    out_ext = nc.declare_dram_parameter("out", [NSTREAM, F_PER, QPOS],
                                        FP32, isOutput=True)

    # SBUF
    w_sb = nc.alloc_sbuf_tensor("w_sb", [128, WCOLS + 1], FP32)
    # stationary: per-filter 8-col blocks (col j = +/-2) at [8j:8j+8],
    # all(-1) block at [64:72] (x sums), zeros at [72:80] (prologue)
    stat = nc.alloc_sbuf_tensor("stat", [128, 8 * F_PER + 16], BF16)
    zmov = nc.alloc_sbuf_tensor("zmov", [128, BANKC], BF16)
    xch = [nc.alloc_sbuf_tensor(f"xc{k}", [128, POS], BF16)
           for k in range(NCHUNK)]
    vring = [nc.alloc_sbuf_tensor(f"vb{r}", [128, POS], BF16)
             for r in range(RING_V)]
    sring = [nc.alloc_sbuf_tensor(f"sb{r}", [128, POS], BF16)
             for r in range(RING_S)]
    osb = nc.alloc_sbuf_tensor("osb", [128, QPOS], FP32)

    # PSUM: 8 banks; stream c owns banks {2c, 2c+1} = cols
    # [1024c, 1024c+784) and computes ALL 8 filters (rows 32c..32c+8)
    # for its private position quarter [784c, 784c+784).  Streams never
    # share a bank: concurrent accumulating matmuls on a shared bank
    # corrupt it (observed on HW).
    psum = nc.alloc_psum_tensor("ps", [128, 8 * BANKC], FP32)

    units = [(j, k) for k in range(NCHUNK) for j in range(F_PER)]
    prod = {}   # (j, k) -> ("v"|"s", producer-local index)
    nv = ns = 0
    for (j, k) in units:
        if j in ACT_F:
            prod[(j, k)] = ("s", ns)
            ns += 1
        else:
            prod[(j, k)] = ("v", nv)
            nv += 1
    NV, NS = nv, ns

    with (
        nc.Block() as block,
        nc.semaphore("w_sem") as w_sem,
        nc.semaphore("x0_sem") as x0_sem,
        nc.semaphore("x1_sem") as x1_sem,
        nc.semaphore("x2_sem") as x2_sem,
        nc.semaphore("x3_sem") as x3_sem,
        nc.semaphore("x4_sem") as x4_sem,
        nc.semaphore("out_sem") as out_sem,
        nc.semaphore("init_sem") as init_sem,
        nc.semaphore("dve_sem") as dve_sem,
        nc.semaphore("actp_sem") as actp_sem,
        nc.semaphore("pe_v_sem") as pe_v_sem,
        nc.semaphore("pe_s_sem") as pe_s_sem,
        nc.semaphore("pe_all_sem") as pe_all_sem,
        nc.semaphore("evac_sem") as evac_sem,
        nc.semaphore("evac2_sem") as evac2_sem,
    ):
        xsem = [x0_sem, x1_sem, x2_sem, x3_sem, x4_sem]
        xthr = [128, 64] + [32] * (NCHUNK - 2)  # 8/4/2-way splits

        @block.sync
        def _(sync: bass.BassEngine):
            # input DMAs are descriptor-rate-bound (~0.5us per 6.3KB
            # partition row per queue), so split hot chunks across many
            # queues; chunk 0 gates the whole pipeline.
            for q in range(8):
                sync.dma_start(
                    out=xch[0][16 * q:16 * (q + 1), :],
                    in_=x_ext[0, 16 * q:16 * (q + 1), :]).then_inc(
                        x0_sem, 16)
            sync.dma_start(out=w_sb[:], in_=w_ext[:]).then_inc(w_sem, 16)
            for q in range(4):
                sync.dma_start(
                    out=xch[1][32 * q:32 * (q + 1), :],
                    in_=x_ext[1, 32 * q:32 * (q + 1), :]).then_inc(
                        x1_sem, 16)
            for k in range(2, NCHUNK):
                for q in range(2):
                    sync.dma_start(
                        out=xch[k][64 * q:64 * (q + 1), :],
                        in_=x_ext[k, 64 * q:64 * (q + 1), :]).then_inc(
                            xsem[k], 16)
            # final output store: out[f, 784c + p] = osb[32c + f, p];
            # one plain 2-D DMA per stream (a single 3-D gather AP gets
            # mangled by the DMA AP optimizer); each goes as soon as its
            # evacuating engine finishes that stream
            for i, c in enumerate((0, 2, 1, 3)):
                sync.wait_ge(evac_sem if c < 2 else evac2_sem, 1 + (c % 2))
                sync.dma_start(
                    out=out_ext[c],
                    in_=osb[32 * c:32 * c + F_PER, :],
                ).then_inc(out_sem, 16)
            sync.wait_ge(out_sem, 16 * NSTREAM)

        @block.vector
        def _(vector: bass.BassEngine):
            # stationary: block j (cols 8j..8j+8): col j = +/-2;
            # x-sum block (cols 64:72) = -1; prologue cols 72:80 = 0
            vector.memset(stat[:], 0.0)
            for j in range(F_PER):
                val = -2.0 if j in ACT_F else 2.0
                vector.memset(stat[:, 8 * j + j:8 * j + j + 1], val)
            vector.memset(stat[:, 8 * F_PER:8 * F_PER + 8], -1.0)
            last = vector.memset(zmov[:], 0.0)
            last.then_inc(init_sem, 1)
            vector.wait_ge(w_sem, 16)
            seen = set()
            for (j, k) in units:
                kind, r = prod[(j, k)]
                if kind != "v":
                    continue
                if k not in seen:
                    seen.add(k)
                    vector.wait_ge(xsem[k], xthr[k])
                if r >= RING_V:
                    vector.wait_ge(pe_v_sem, r - RING_V + 1)
                col = k * F_PER + j
                vector.tensor_scalar(
                    out=vring[r % RING_V][:], in0=xch[k][:],
                    scalar1=w_sb[:, col:col + 1], scalar2=0.0,
                    op0=mybir.AluOpType.subtract,
                    op1=mybir.AluOpType.min,
                ).then_inc(dve_sem, 1)
            # evacuate streams 2..3 (psum rows 32c..32c+8 + sum_d W)
            vector.wait_ge(pe_all_sem, 1)
            for c in range(2, NSTREAM):
                vector.tensor_scalar(
                    out=osb[32 * c:32 * c + F_PER, :],
                    in0=psum[32 * c:32 * c + F_PER,
                             1024 * c:1024 * c + QPOS],
                    scalar1=w_sb[32 * c:32 * c + F_PER, WCOLS:WCOLS + 1],
                    scalar2=None,
                    op0=mybir.AluOpType.add,
                ).then_inc(evac2_sem, 1)

        @block.scalar
        def _(scalar: bass.BassEngine):
            # touch the Relu table first so the one-time ACT table load
            # overlaps the input DMAs instead of the first real unit
            scalar.activation(osb[0:1, 0:1], zmov[0:1, 0:1],
                              mybir.ActivationFunctionType.Relu,
                              bias=0.0, scale=1.0)
            scalar.wait_ge(w_sem, 16)
            seen = set()
            for (j, k) in units:
                kind, r = prod[(j, k)]
                if kind != "s":
                    continue
                if k not in seen:
                    seen.add(k)
                    scalar.wait_ge(xsem[k], xthr[k])
                if r >= RING_S:
                    scalar.wait_ge(pe_s_sem, r - RING_S + 1)
                col = k * F_PER + j
                scalar.activation(
                    sring[r % RING_S][:], xch[k][:],
                    mybir.ActivationFunctionType.Relu,
                    bias=w_sb[:, col:col + 1], scale=-1.0,
                ).then_inc(actp_sem, 1)
            # evacuate streams 0..1
            scalar.wait_ge(pe_all_sem, 1)
            for c in range(0, 2):
                scalar.activation(
                    osb[32 * c:32 * c + F_PER, :],
                    psum[32 * c:32 * c + F_PER, 1024 * c:1024 * c + QPOS],
                    mybir.ActivationFunctionType.Identity,
                    bias=w_sb[32 * c:32 * c + F_PER, WCOLS:WCOLS + 1],
                    scale=1.0,
                ).then_inc(evac_sem, 1)

        @block.tensor
        def _(tensor: bass.BassEngine):
            tensor.wait_ge(init_sem, 1)  # stat + zmov memsets done
            # prologue: zero each stream's psum rows in its private
            # banks; each bank's first matmul carries start=True
            for c in range(NSTREAM):
                for (so, sw) in SUB:
                    tensor.matmul(
                        psum[32 * c:32 * c + F_PER,
                             1024 * c + so:1024 * c + so + sw],
                        stat[:, 8 * F_PER + 8:8 * F_PER + 16],
                        zmov[:, 0:sw],
                        start=True, stop=False, skip_group_check=True,
                        tile_position=(0, 32 * c),
                    )
            kdone = set()
            for (j, k) in units:
                if k not in kdone:
                    kdone.add(k)
                    tensor.wait_ge(xsem[k], xthr[k])
                    # -sum_d x for chunk k (all filters) on every stream
                    for c in range(NSTREAM):
                        for (so, sw) in SUB:
                            tensor.matmul(
                                psum[32 * c:32 * c + F_PER,
                                     1024 * c + so:1024 * c + so + sw],
                                stat[:, 8 * F_PER:8 * F_PER + 8],
                                xch[k][:, QPOS * c + so:QPOS * c + so + sw],
                                start=False, stop=False,
                                skip_group_check=True,
                                tile_position=(0, 32 * c),
                            )
                kind, r = prod[(j, k)]
                if kind == "v":
                    tensor.wait_ge(dve_sem, r + 1)
                    a = vring[r % RING_V]
                else:
                    tensor.wait_ge(actp_sem, r + 1)
                    a = sring[r % RING_S]
                is_last = (j, k) == units[-1]
                for c in range(NSTREAM):
                    for si, (so, sw) in enumerate(SUB):
                        fin = c == NSTREAM - 1 and si == len(SUB) - 1
                        mm = tensor.matmul(
                            psum[32 * c:32 * c + F_PER,
                                 1024 * c + so:1024 * c + so + sw],
                            stat[:, 8 * j:8 * j + 8],
                            a[:, QPOS * c + so:QPOS * c + so + sw],
                            start=False, stop=is_last and fin,
                            skip_group_check=True,
                            tile_position=(0, 32 * c),
                        )
                        if fin:
                            mm.then_inc(
                                pe_v_sem if kind == "v" else pe_s_sem, 1)
            # all matmuls retired in pc order once the last one is done
            tensor.wait_ge(pe_v_sem, NV)
            tensor.wait_ge(pe_s_sem, NS)
            tensor.nop().then_inc(pe_all_sem, 1)

    return nc


def _prep_inputs(x: np.ndarray, W: np.ndarray):
    x = np.asarray(x, dtype=np.float32)
    W = np.asarray(W, dtype=np.float32)
    # Host im2col in (kh, kw, c) d-order -> zero-padded (5, 128, POS) bf16
    xp = np.zeros((C, N, H + 2, W_SP + 2), np.float32)
    xp[:, :, PAD:PAD + H, PAD:PAD + W_SP] = x.transpose(1, 0, 2, 3)
    xc = np.zeros((NCHUNK * 128, POS), np.float32)
    for b in range(KK * KK):
        kh, kw = divmod(b, KK)
        xc[64 * b:64 * (b + 1), :] = (
            xp[:, :, kh:kh + H, kw:kw + W_SP].reshape(C, POS))
    xpad = xc.reshape(NCHUNK, 128, POS).astype(ml_dtypes.bfloat16)
    # W_col in (kh, kw, c) d-order: (F, 576)
    Wp = W.transpose(0, 2, 3, 1).reshape(F, KK * KK * C)
    wtiles = []
    for i in range(N_CORES):
        wt = np.zeros((128, WCOLS + 1), np.float32)
        for k in range(NCHUNK):
            dd = min(128, D - 128 * k)
            blk = Wp[F_PER * i:F_PER * (i + 1), 128 * k:128 * k + dd].T
            wt[:dd, k * F_PER:(k + 1) * F_PER] = blk
        sw = Wp[F_PER * i:F_PER * (i + 1), :].sum(axis=1)
        for c in range(NSTREAM):
            wt[32 * c:32 * c + F_PER, WCOLS] = sw
        wtiles.append(wt)
    return xpad, wtiles


_CACHED_NC = None
LAST_RESULT = None  # BassKernelResults of the most recent run (for test.py)


def kernel(x: np.ndarray, W: np.ndarray, _trace: bool = False) -> np.ndarray:
    global _CACHED_NC, LAST_RESULT
    xpad, wtiles = _prep_inputs(x, W)
    if _CACHED_NC is None:
        _CACHED_NC = build_bass()
    nc = _CACHED_NC
    in_maps = [{"xcol": xpad, "wcols": wtiles[i]} for i in range(N_CORES)]
    res = run_bass_kernel_spmd(nc, in_maps, core_ids=list(range(N_CORES)),
                               trace=_trace)
    LAST_RESULT = res
    outs = [np.asarray(res.results[i]["out"], dtype=np.float32)
            .transpose(1, 0, 2).reshape(F_PER, POS)
            for i in range(N_CORES)]
    o = np.concatenate(outs, axis=0)                    # (64, 3136)
    o = (o.reshape(F, N, P).transpose(1, 0, 2)
          .reshape(N, F, H, W_SP).astype(np.float32))
    return o


# revision 30
# speedup vs baseline: 1.0288x; 1.0288x over previous
"""AdderNet 2D conv (L1-distance "convolution") on 8 TRN2 NeuronCores.

Reference computation:
    X_col = unfold(x, k=3, stride=1, pad=1)      # (N, D, P)  D=576, P=196
    out[n, f, p] = -sum_d |W_col[f, d] - X_col[n, d, p]|

Distribution: filter-parallel — core i computes filters f in [8i, 8i+8)
for the FULL batch (no collectives; host concatenates filter slices).
This makes the per-instruction free dim N*P = 3136, which amortizes
per-instruction overhead far better than batch-parallel (392).

Per-core algorithm (raw Bass; this walrus encodes only ONE inline
sync-wait per instruction, so Tile's auto-semaphores don't compile —
standalone wait_ge instructions are used instead):

  -sum_d |x-w|  =  -sum_d x  + sum_d w  + 2*sum_d min(x-w, 0)

  - Host im2col: d (patch dim, 576, (kh,kw,c)-ordered) on SBUF
    partitions, 5 zero-padded chunks of 128, shipped as dense
    (128, 3136) bf16 tiles (window DMAs would shatter into 28-byte
    descriptors and starve everything).
  - Per (filter, chunk) unit, ONE elementwise instruction:
      VectorE: tensor_scalar(op0=sub W[f,.], op1=min 0)  -> min(x-w, 0)
      ScalarE: activation(Relu, scale=-1, bias=W[f,.])   -> relu(w-x)
    (no encodable fused abs exists on the Vector engine in this ISA;
    the f's are split between the engines to balance them).
  - TensorE reduces over partitions into PSUM, 4-way column-tiled:
    filter pair (2c, 2c+1) lives on PSUM partitions {32c, 32c+1} and
    its matmuls run on array column-group c — four concurrent moving
    streams (tile_position=(0, 32c)).  Stationary column = +2 (vector
    units) / -2 (scalar units); an all(-1) 2-col stationary block
    accumulates -sum_d x once per (chunk, stream).  PSUM is one
    (128, 7*512) tensor: matmul moving slices of 512 columns map
    exactly onto banks.  A zero-fill prologue (start=True, zero
    moving) initializes every bank's has_written bits so all real
    matmuls can accumulate without ordering hazards across streams.
  - Evacuation: per stream, psum rows {32c, 32c+1} + bias sum_d W[f]
    (activation bias / tensor_scalar add) -> osb rows {32c, 32c+1};
    streams 0-1 on ScalarE, 2-3 on VectorE.  The out DMA gathers the
    8 scattered osb rows with a 2-level partition access pattern.

kernel(x, W) accepts the FULL inputs and returns the FULL output.
"""

import os

import numpy as np
import ml_dtypes

import concourse.bass as bass
from concourse import mybir
from concourse.bass_utils import run_bass_kernel_spmd

# Problem constants (hardcoded per harness rules)
N, C, H, W_SP = 16, 64, 14, 14
F = 64
KK = 3
PAD = 1
P = H * W_SP            # 196 output positions per image
POS = N * P             # 3136 total positions
D = C * KK * KK         # 576
N_CORES = 8
F_PER = F // N_CORES    # 8 filters per core
NCHUNK = 5              # ceil(576 / 128) d-chunks
NSTREAM = 4             # TensorE column-tiling streams
QPOS = POS // NSTREAM   # 784 positions per stream (its private quarter)
BANKC = 512             # psum bank capacity in f32
SUB = [(0, 512), (512, 272)]  # per-stream sub-slices (2 private banks)
RING_V = 8              # vector-produced tile ring
RING_S = 3              # scalar-produced tile ring

FP32 = mybir.dt.float32
BF16 = mybir.dt.bfloat16

# Filters handled by the Scalar engine (the rest go to VectorE).
ACT_F = tuple(
    int(t) for t in os.environ.get("ADDER_ACT_F", "3,7").split(",") if t != ""
)

WCOLS = NCHUNK * F_PER  # 40 W columns (col = k*8+j)


def build_bass():
    nc = bass.Bass()

    x_ext = nc.declare_dram_parameter("xcol", [NCHUNK, 128, POS], BF16,
                                      isOutput=False)
    # cols 0:40 = W columns (col = k*8 + j); col 40 row (32*(j//2)+j%2) =
    # sum_d W[f_j]
    w_ext = nc.declare_dram_parameter("wcols", [128, WCOLS + 1], FP32,
                                      isOutput=False)
    out_ext = nc.declare_dram_parameter("out", [NSTREAM, F_PER, QPOS],
                                        FP32, isOutput=True)

    # SBUF
    w_sb = nc.alloc_sbuf_tensor("w_sb", [128, WCOLS + 1], FP32)
    # stationary: per-filter 8-col blocks (col j = +/-2) at [8j:8j+8],
    # all(-1) block at [64:72] (x sums), zeros at [72:80] (prologue)
    stat = nc.alloc_sbuf_tensor("stat", [128, 8 * F_PER + 16], BF16)
    zmov = nc.alloc_sbuf_tensor("zmov", [128, BANKC], BF16)
    xch = [nc.alloc_sbuf_tensor(f"xc{k}", [128, POS], BF16)
           for k in range(NCHUNK)]
    vring = [nc.alloc_sbuf_tensor(f"vb{r}", [128, POS], BF16)
             for r in range(RING_V)]
    sring = [nc.alloc_sbuf_tensor(f"sb{r}", [128, POS], BF16)
             for r in range(RING_S)]
    osb = nc.alloc_sbuf_tensor("osb", [128, QPOS], FP32)

    # PSUM: 8 banks; stream c owns banks {2c, 2c+1} = cols
    # [1024c, 1024c+784) and computes ALL 8 filters (rows 32c..32c+8)
    # for its private position quarter [784c, 784c+784).  Streams never
    # share a bank: concurrent accumulating matmuls on a shared bank
    # corrupt it (observed on HW).
    psum = nc.alloc_psum_tensor("ps", [128, 8 * BANKC], FP32)

    units = [(j, k) for k in range(NCHUNK) for j in range(F_PER)]
    prod = {}   # (j, k) -> ("v"|"s", producer-local index)
    nv = ns = 0
    for (j, k) in units:
        if j in ACT_F:
            prod[(j, k)] = ("s", ns)
            ns += 1
        else:
            prod[(j, k)] = ("v", nv)
            nv += 1
    NV, NS = nv, ns

    with (
        nc.Block() as block,
        nc.semaphore("w_sem") as w_sem,
        nc.semaphore("x0_sem") as x0_sem,
        nc.semaphore("x1_sem") as x1_sem,
        nc.semaphore("x2_sem") as x2_sem,
        nc.semaphore("x3_sem") as x3_sem,
        nc.semaphore("x4_sem") as x4_sem,
        nc.semaphore("out_sem") as out_sem,
        nc.semaphore("init_sem") as init_sem,
        nc.semaphore("dve_sem") as dve_sem,
        nc.semaphore("actp_sem") as actp_sem,
        nc.semaphore("pe_v_sem") as pe_v_sem,
        nc.semaphore("pe_s_sem") as pe_s_sem,
        nc.semaphore("pe_all_sem") as pe_all_sem,
        nc.semaphore("evac_sem") as evac_sem,
        nc.semaphore("evac2_sem") as evac2_sem,
    ):
        xsem = [x0_sem, x1_sem, x2_sem, x3_sem, x4_sem]
        xthr = [64, 64] + [32] * (NCHUNK - 2)  # 4/4/2-way splits

        @block.sync
        def _(sync: bass.BassEngine):
            # input DMAs are descriptor-rate-bound; chunk 0 gates the
            # whole pipeline, so give it exclusive DMA bandwidth first,
            # then stream the rest (they complete well before use).
            for q in range(4):
                sync.dma_start(
                    out=xch[0][32 * q:32 * (q + 1), :],
                    in_=x_ext[0, 32 * q:32 * (q + 1), :]).then_inc(
                        x0_sem, 16)
            sync.wait_ge(x0_sem, 64)
            sync.dma_start(out=w_sb[:], in_=w_ext[:]).then_inc(w_sem, 16)
            for q in range(4):
                sync.dma_start(
                    out=xch[1][32 * q:32 * (q + 1), :],
                    in_=x_ext[1, 32 * q:32 * (q + 1), :]).then_inc(
                        x1_sem, 16)
            for k in range(2, NCHUNK):
                for q in range(2):
                    sync.dma_start(
                        out=xch[k][64 * q:64 * (q + 1), :],
                        in_=x_ext[k, 64 * q:64 * (q + 1), :]).then_inc(
                            xsem[k], 16)
            # final output store: out[f, 784c + p] = osb[32c + f, p];
            # one plain 2-D DMA per stream (a single 3-D gather AP gets
            # mangled by the DMA AP optimizer); each goes as soon as its
            # evacuating engine finishes that stream
            for i, c in enumerate((0, 2, 1, 3)):
                sync.wait_ge(evac_sem if c < 2 else evac2_sem, 1 + (c % 2))
                sync.dma_start(
                    out=out_ext[c],
                    in_=osb[32 * c:32 * c + F_PER, :],
                ).then_inc(out_sem, 16)
            sync.wait_ge(out_sem, 16 * NSTREAM)

        @block.vector
        def _(vector: bass.BassEngine):
            # stationary: block j (cols 8j..8j+8): col j = +/-2;
            # x-sum block (cols 64:72) = -1; prologue cols 72:80 = 0
            vector.memset(stat[:], 0.0)
            for j in range(F_PER):
                val = -2.0 if j in ACT_F else 2.0
                vector.memset(stat[:, 8 * j + j:8 * j + j + 1], val)
            vector.memset(stat[:, 8 * F_PER:8 * F_PER + 8], -1.0)
            last = vector.memset(zmov[:], 0.0)
            last.then_inc(init_sem, 1)
            vector.wait_ge(w_sem, 16)
            seen = set()
            for (j, k) in units:
                kind, r = prod[(j, k)]
                if kind != "v":
                    continue
                if k not in seen:
                    seen.add(k)
                    vector.wait_ge(xsem[k], xthr[k])
                if r >= RING_V:
                    vector.wait_ge(pe_v_sem, r - RING_V + 1)
                col = k * F_PER + j
                vector.tensor_scalar(
                    out=vring[r % RING_V][:], in0=xch[k][:],
                    scalar1=w_sb[:, col:col + 1], scalar2=0.0,
                    op0=mybir.AluOpType.subtract,
                    op1=mybir.AluOpType.min,
                ).then_inc(dve_sem, 1)
            # evacuate streams 2..3 (psum rows 32c..32c+8 + sum_d W)
            vector.wait_ge(pe_all_sem, 1)
            for c in range(2, NSTREAM):
                vector.tensor_scalar(
                    out=osb[32 * c:32 * c + F_PER, :],
                    in0=psum[32 * c:32 * c + F_PER,
                             1024 * c:1024 * c + QPOS],
                    scalar1=w_sb[32 * c:32 * c + F_PER, WCOLS:WCOLS + 1],
                    scalar2=None,
                    op0=mybir.AluOpType.add,
                ).then_inc(evac2_sem, 1)

        @block.scalar
        def _(scalar: bass.BassEngine):
            # touch the Relu table first so the one-time ACT table load
            # overlaps the input DMAs instead of the first real unit
            scalar.activation(osb[0:1, 0:1], zmov[0:1, 0:1],
                              mybir.ActivationFunctionType.Relu,
                              bias=0.0, scale=1.0)
            scalar.wait_ge(w_sem, 16)
            seen = set()
            for (j, k) in units:
                kind, r = prod[(j, k)]
                if kind != "s":
                    continue
                if k not in seen:
                    seen.add(k)
                    scalar.wait_ge(xsem[k], xthr[k])
                if r >= RING_S:
                    scalar.wait_ge(pe_s_sem, r - RING_S + 1)
                col = k * F_PER + j
                scalar.activation(
                    sring[r % RING_S][:], xch[k][:],
                    mybir.ActivationFunctionType.Relu,
                    bias=w_sb[:, col:col + 1], scale=-1.0,
                ).then_inc(actp_sem, 1)
            # evacuate streams 0..1
            scalar.wait_ge(pe_all_sem, 1)
            for c in range(0, 2):
                scalar.activation(
                    osb[32 * c:32 * c + F_PER, :],
                    psum[32 * c:32 * c + F_PER, 1024 * c:1024 * c + QPOS],
                    mybir.ActivationFunctionType.Identity,
                    bias=w_sb[32 * c:32 * c + F_PER, WCOLS:WCOLS + 1],
                    scale=1.0,
                ).then_inc(evac_sem, 1)

        @block.tensor
        def _(tensor: bass.BassEngine):
            tensor.wait_ge(init_sem, 1)  # stat + zmov memsets done
            # prologue: zero each stream's psum rows in its private
            # banks; each bank's first matmul carries start=True
            for c in range(NSTREAM):
                for (so, sw) in SUB:
                    tensor.matmul(
                        psum[32 * c:32 * c + F_PER,
                             1024 * c + so:1024 * c + so + sw],
                        stat[:, 8 * F_PER + 8:8 * F_PER + 16],
                        zmov[:, 0:sw],
                        start=True, stop=False, skip_group_check=True,
                        tile_position=(0, 32 * c),
                    )
            kdone = set()
            for (j, k) in units:
                if k not in kdone:
                    kdone.add(k)
                    tensor.wait_ge(xsem[k], xthr[k])
                    # -sum_d x for chunk k (all filters) on every stream
                    for c in range(NSTREAM):
                        for (so, sw) in SUB:
                            tensor.matmul(
                                psum[32 * c:32 * c + F_PER,
                                     1024 * c + so:1024 * c + so + sw],
                                stat[:, 8 * F_PER:8 * F_PER + 8],
                                xch[k][:, QPOS * c + so:QPOS * c + so + sw],
                                start=False, stop=False,
                                skip_group_check=True,
                                tile_position=(0, 32 * c),
                            )
                kind, r = prod[(j, k)]
                if kind == "v":
                    tensor.wait_ge(dve_sem, r + 1)
                    a = vring[r % RING_V]
                else:
                    tensor.wait_ge(actp_sem, r + 1)
                    a = sring[r % RING_S]
                is_last = (j, k) == units[-1]
                for c in range(NSTREAM):
                    for si, (so, sw) in enumerate(SUB):
                        fin = c == NSTREAM - 1 and si == len(SUB) - 1
                        mm = tensor.matmul(
                            psum[32 * c:32 * c + F_PER,
                                 1024 * c + so:1024 * c + so + sw],
                            stat[:, 8 * j:8 * j + 8],
                            a[:, QPOS * c + so:QPOS * c + so + sw],
                            start=False, stop=is_last and fin,
                            skip_group_check=True,
                            tile_position=(0, 32 * c),
                        )
                        if fin:
                            mm.then_inc(
                                pe_v_sem if kind == "v" else pe_s_sem, 1)
            # all matmuls retired in pc order once the last one is done
            tensor.wait_ge(pe_v_sem, NV)
            tensor.wait_ge(pe_s_sem, NS)
            tensor.nop().then_inc(pe_all_sem, 1)

    return nc


def _prep_inputs(x: np.ndarray, W: np.ndarray):
    x = np.asarray(x, dtype=np.float32)
    W = np.asarray(W, dtype=np.float32)
    # Host im2col in (kh, kw, c) d-order -> zero-padded (5, 128, POS) bf16
    xp = np.zeros((C, N, H + 2, W_SP + 2), np.float32)
    xp[:, :, PAD:PAD + H, PAD:PAD + W_SP] = x.transpose(1, 0, 2, 3)
    xc = np.zeros((NCHUNK * 128, POS), np.float32)
    for b in range(KK * KK):
        kh, kw = divmod(b, KK)
        xc[64 * b:64 * (b + 1), :] = (
            xp[:, :, kh:kh + H, kw:kw + W_SP].reshape(C, POS))
    xpad = xc.reshape(NCHUNK, 128, POS).astype(ml_dtypes.bfloat16)
    # W_col in (kh, kw, c) d-order: (F, 576)
    Wp = W.transpose(0, 2, 3, 1).reshape(F, KK * KK * C)
    wtiles = []
    for i in range(N_CORES):
        wt = np.zeros((128, WCOLS + 1), np.float32)
        for k in range(NCHUNK):
            dd = min(128, D - 128 * k)
            blk = Wp[F_PER * i:F_PER * (i + 1), 128 * k:128 * k + dd].T
            wt[:dd, k * F_PER:(k + 1) * F_PER] = blk
        sw = Wp[F_PER * i:F_PER * (i + 1), :].sum(axis=1)
        for c in range(NSTREAM):
            wt[32 * c:32 * c + F_PER, WCOLS] = sw
        wtiles.append(wt)
    return xpad, wtiles


_CACHED_NC = None
LAST_RESULT = None  # BassKernelResults of the most recent run (for test.py)


def kernel(x: np.ndarray, W: np.ndarray, _trace: bool = False) -> np.ndarray:
    global _CACHED_NC, LAST_RESULT
    xpad, wtiles = _prep_inputs(x, W)
    if _CACHED_NC is None:
        _CACHED_NC = build_bass()
    nc = _CACHED_NC
    in_maps = [{"xcol": xpad, "wcols": wtiles[i]} for i in range(N_CORES)]
    res = run_bass_kernel_spmd(nc, in_maps, core_ids=list(range(N_CORES)),
                               trace=_trace)
    LAST_RESULT = res
    outs = [np.asarray(res.results[i]["out"], dtype=np.float32)
            .transpose(1, 0, 2).reshape(F_PER, POS)
            for i in range(N_CORES)]
    o = np.concatenate(outs, axis=0)                    # (64, 3136)
    o = (o.reshape(F, N, P).transpose(1, 0, 2)
          .reshape(N, F, H, W_SP).astype(np.float32))
    return o


# revision 31
# speedup vs baseline: 1.0516x; 1.0222x over previous
"""AdderNet 2D conv (L1-distance "convolution") on 8 TRN2 NeuronCores.

Reference computation:
    X_col = unfold(x, k=3, stride=1, pad=1)      # (N, D, P)  D=576, P=196
    out[n, f, p] = -sum_d |W_col[f, d] - X_col[n, d, p]|

Distribution: filter-parallel — core i computes filters f in [8i, 8i+8)
for the FULL batch (no collectives; host concatenates filter slices).
This makes the per-instruction free dim N*P = 3136, which amortizes
per-instruction overhead far better than batch-parallel (392).

Per-core algorithm (raw Bass; this walrus encodes only ONE inline
sync-wait per instruction, so Tile's auto-semaphores don't compile —
standalone wait_ge instructions are used instead):

  -sum_d |x-w|  =  -sum_d x  + sum_d w  + 2*sum_d min(x-w, 0)

  - Host im2col: d (patch dim, 576, (kh,kw,c)-ordered) on SBUF
    partitions, 5 zero-padded chunks of 128, shipped as dense
    (128, 3136) bf16 tiles (window DMAs would shatter into 28-byte
    descriptors and starve everything).
  - Per (filter, chunk) unit, ONE elementwise instruction:
      VectorE: tensor_scalar(op0=sub W[f,.], op1=min 0)  -> min(x-w, 0)
      ScalarE: activation(Relu, scale=-1, bias=W[f,.])   -> relu(w-x)
    (no encodable fused abs exists on the Vector engine in this ISA;
    the f's are split between the engines to balance them).
  - TensorE reduces over partitions into PSUM, 4-way column-tiled:
    filter pair (2c, 2c+1) lives on PSUM partitions {32c, 32c+1} and
    its matmuls run on array column-group c — four concurrent moving
    streams (tile_position=(0, 32c)).  Stationary column = +2 (vector
    units) / -2 (scalar units); an all(-1) 2-col stationary block
    accumulates -sum_d x once per (chunk, stream).  PSUM is one
    (128, 7*512) tensor: matmul moving slices of 512 columns map
    exactly onto banks.  A zero-fill prologue (start=True, zero
    moving) initializes every bank's has_written bits so all real
    matmuls can accumulate without ordering hazards across streams.
  - Evacuation: per stream, psum rows {32c, 32c+1} + bias sum_d W[f]
    (activation bias / tensor_scalar add) -> osb rows {32c, 32c+1};
    streams 0-1 on ScalarE, 2-3 on VectorE.  The out DMA gathers the
    8 scattered osb rows with a 2-level partition access pattern.

kernel(x, W) accepts the FULL inputs and returns the FULL output.
"""

import os

import numpy as np
import ml_dtypes

import concourse.bass as bass
from concourse import mybir
from concourse.bass_utils import run_bass_kernel_spmd

# Problem constants (hardcoded per harness rules)
N, C, H, W_SP = 16, 64, 14, 14
F = 64
KK = 3
PAD = 1
P = H * W_SP            # 196 output positions per image
POS = N * P             # 3136 total positions
D = C * KK * KK         # 576
N_CORES = 8
F_PER = F // N_CORES    # 8 filters per core
NCHUNK = 5              # ceil(576 / 128) d-chunks
NSTREAM = 4             # TensorE column-tiling streams
QPOS = POS // NSTREAM   # 784 positions per stream (its private quarter)
BANKC = 512             # psum bank capacity in f32
SUB = [(0, 512), (512, 272)]  # per-stream sub-slices (2 private banks)
RING_V = 8              # vector-produced tile ring
RING_S = 3              # scalar-produced tile ring

FP32 = mybir.dt.float32
BF16 = mybir.dt.bfloat16

# Filters handled by the Scalar engine (the rest go to VectorE).
ACT_F = tuple(
    int(t) for t in os.environ.get("ADDER_ACT_F", "3,7").split(",") if t != ""
)

WCOLS = NCHUNK * F_PER  # 40 W columns (col = k*8+j)


def build_bass():
    nc = bass.Bass()

    x_ext = nc.declare_dram_parameter("xcol", [NCHUNK, 128, POS], BF16,
                                      isOutput=False)
    # cols 0:40 = W columns (col = k*8 + j); col 40 row (32*(j//2)+j%2) =
    # sum_d W[f_j]
    w_ext = nc.declare_dram_parameter("wcols", [128, WCOLS + 1], FP32,
                                      isOutput=False)
    out_ext = nc.declare_dram_parameter("out", [NSTREAM, F_PER, QPOS],
                                        FP32, isOutput=True)

    # SBUF
    w_sb = nc.alloc_sbuf_tensor("w_sb", [128, WCOLS + 1], FP32)
    # stationary: per-filter 8-col blocks (col j = +/-2) at [8j:8j+8],
    # all(-1) block at [64:72] (x sums), zeros at [72:80] (prologue)
    stat = nc.alloc_sbuf_tensor("stat", [128, 8 * F_PER + 16], BF16)
    zmov = nc.alloc_sbuf_tensor("zmov", [128, BANKC], BF16)
    xch = [nc.alloc_sbuf_tensor(f"xc{k}", [128, POS], BF16)
           for k in range(NCHUNK)]
    vring = [nc.alloc_sbuf_tensor(f"vb{r}", [128, POS], BF16)
             for r in range(RING_V)]
    sring = [nc.alloc_sbuf_tensor(f"sb{r}", [128, POS], BF16)
             for r in range(RING_S)]
    osb = nc.alloc_sbuf_tensor("osb", [128, QPOS], FP32)

    # PSUM: 8 banks; stream c owns banks {2c, 2c+1} = cols
    # [1024c, 1024c+784) and computes ALL 8 filters (rows 32c..32c+8)
    # for its private position quarter [784c, 784c+784).  Streams never
    # share a bank: concurrent accumulating matmuls on a shared bank
    # corrupt it (observed on HW).
    psum = nc.alloc_psum_tensor("ps", [128, 8 * BANKC], FP32)

    units = [(j, k) for k in range(NCHUNK) for j in range(F_PER)]
    prod = {}   # (j, k) -> ("v"|"s", producer-local index)
    nv = ns = 0
    for (j, k) in units:
        if j in ACT_F:
            prod[(j, k)] = ("s", ns)
            ns += 1
        else:
            prod[(j, k)] = ("v", nv)
            nv += 1
    NV, NS = nv, ns

    with (
        nc.Block() as block,
        nc.semaphore("w_sem") as w_sem,
        nc.semaphore("x0_sem") as x0_sem,
        nc.semaphore("x1_sem") as x1_sem,
        nc.semaphore("x2_sem") as x2_sem,
        nc.semaphore("x3_sem") as x3_sem,
        nc.semaphore("x4_sem") as x4_sem,
        nc.semaphore("out_sem") as out_sem,
        nc.semaphore("init_sem") as init_sem,
        nc.semaphore("dve_sem") as dve_sem,
        nc.semaphore("actp_sem") as actp_sem,
        nc.semaphore("pe_v_sem") as pe_v_sem,
        nc.semaphore("pe_s_sem") as pe_s_sem,
        nc.semaphore("pe_all_sem") as pe_all_sem,
        nc.semaphore("evac_sem") as evac_sem,
        nc.semaphore("evac2_sem") as evac2_sem,
    ):
        xsem = [x0_sem, x1_sem, x2_sem, x3_sem, x4_sem]
        xthr = [64, 64] + [32] * (NCHUNK - 2)  # 4/4/2-way splits

        @block.sync
        def _(sync: bass.BassEngine):
            # input DMAs are descriptor-rate-bound; chunk 0 gates the
            # whole pipeline, so give it exclusive DMA bandwidth first,
            # then stream the rest (they complete well before use).
            sync.dma_start(out=w_sb[:], in_=w_ext[:]).then_inc(w_sem, 16)
            for q in range(4):
                sync.dma_start(
                    out=xch[0][32 * q:32 * (q + 1), :],
                    in_=x_ext[0, 32 * q:32 * (q + 1), :]).then_inc(
                        x0_sem, 16)
            sync.wait_ge(x0_sem, 64)
            for q in range(4):
                sync.dma_start(
                    out=xch[1][32 * q:32 * (q + 1), :],
                    in_=x_ext[1, 32 * q:32 * (q + 1), :]).then_inc(
                        x1_sem, 16)
            for k in range(2, NCHUNK):
                for q in range(2):
                    sync.dma_start(
                        out=xch[k][64 * q:64 * (q + 1), :],
                        in_=x_ext[k, 64 * q:64 * (q + 1), :]).then_inc(
                            xsem[k], 16)
            # final output store: out[f, 784c + p] = osb[32c + f, p];
            # one plain 2-D DMA per stream (a single 3-D gather AP gets
            # mangled by the DMA AP optimizer); each goes as soon as its
            # evacuating engine finishes that stream
            for i, c in enumerate((0, 2, 1, 3)):
                sync.wait_ge(evac_sem if c < 2 else evac2_sem, 1 + (c % 2))
                sync.dma_start(
                    out=out_ext[c],
                    in_=osb[32 * c:32 * c + F_PER, :],
                ).then_inc(out_sem, 16)
            sync.wait_ge(out_sem, 16 * NSTREAM)

        @block.vector
        def _(vector: bass.BassEngine):
            # stationary: block j (cols 8j..8j+8): col j = +/-2;
            # x-sum block (cols 64:72) = -1; prologue cols 72:80 = 0
            vector.memset(stat[:], 0.0)
            for j in range(F_PER):
                val = -2.0 if j in ACT_F else 2.0
                vector.memset(stat[:, 8 * j + j:8 * j + j + 1], val)
            vector.memset(stat[:, 8 * F_PER:8 * F_PER + 8], -1.0)
            last = vector.memset(zmov[:], 0.0)
            last.then_inc(init_sem, 1)
            vector.wait_ge(w_sem, 16)
            seen = set()
            for (j, k) in units:
                kind, r = prod[(j, k)]
                if kind != "v":
                    continue
                if k not in seen:
                    seen.add(k)
                    vector.wait_ge(xsem[k], xthr[k])
                if r >= RING_V:
                    vector.wait_ge(pe_v_sem, r - RING_V + 1)
                col = k * F_PER + j
                vector.tensor_scalar(
                    out=vring[r % RING_V][:], in0=xch[k][:],
                    scalar1=w_sb[:, col:col + 1], scalar2=0.0,
                    op0=mybir.AluOpType.subtract,
                    op1=mybir.AluOpType.min,
                ).then_inc(dve_sem, 1)
            # evacuate streams 2..3 (psum rows 32c..32c+8 + sum_d W)
            vector.wait_ge(pe_all_sem, 1)
            for c in range(2, NSTREAM):
                vector.tensor_scalar(
                    out=osb[32 * c:32 * c + F_PER, :],
                    in0=psum[32 * c:32 * c + F_PER,
                             1024 * c:1024 * c + QPOS],
                    scalar1=w_sb[32 * c:32 * c + F_PER, WCOLS:WCOLS + 1],
                    scalar2=None,
                    op0=mybir.AluOpType.add,
                ).then_inc(evac2_sem, 1)

        @block.scalar
        def _(scalar: bass.BassEngine):
            # touch the Relu table first so the one-time ACT table load
            # overlaps the input DMAs instead of the first real unit
            scalar.activation(osb[0:1, 0:1], zmov[0:1, 0:1],
                              mybir.ActivationFunctionType.Relu,
                              bias=0.0, scale=1.0)
            scalar.wait_ge(w_sem, 16)
            seen = set()
            for (j, k) in units:
                kind, r = prod[(j, k)]
                if kind != "s":
                    continue
                if k not in seen:
                    seen.add(k)
                    scalar.wait_ge(xsem[k], xthr[k])
                if r >= RING_S:
                    scalar.wait_ge(pe_s_sem, r - RING_S + 1)
                col = k * F_PER + j
                scalar.activation(
                    sring[r % RING_S][:], xch[k][:],
                    mybir.ActivationFunctionType.Relu,
                    bias=w_sb[:, col:col + 1], scale=-1.0,
                ).then_inc(actp_sem, 1)
            # evacuate streams 0..1
            scalar.wait_ge(pe_all_sem, 1)
            for c in range(0, 2):
                scalar.activation(
                    osb[32 * c:32 * c + F_PER, :],
                    psum[32 * c:32 * c + F_PER, 1024 * c:1024 * c + QPOS],
                    mybir.ActivationFunctionType.Identity,
                    bias=w_sb[32 * c:32 * c + F_PER, WCOLS:WCOLS + 1],
                    scale=1.0,
                ).then_inc(evac_sem, 1)

        @block.tensor
        def _(tensor: bass.BassEngine):
            tensor.wait_ge(init_sem, 1)  # stat + zmov memsets done
            # prologue: zero each stream's psum rows in its private
            # banks; each bank's first matmul carries start=True
            for c in range(NSTREAM):
                for (so, sw) in SUB:
                    tensor.matmul(
                        psum[32 * c:32 * c + F_PER,
                             1024 * c + so:1024 * c + so + sw],
                        stat[:, 8 * F_PER + 8:8 * F_PER + 16],
                        zmov[:, 0:sw],
                        start=True, stop=False, skip_group_check=True,
                        tile_position=(0, 32 * c),
                    )
            kdone = set()
            for (j, k) in units:
                if k not in kdone:
                    kdone.add(k)
                    tensor.wait_ge(xsem[k], xthr[k])
                    # -sum_d x for chunk k (all filters) on every stream
                    for c in range(NSTREAM):
                        for (so, sw) in SUB:
                            tensor.matmul(
                                psum[32 * c:32 * c + F_PER,
                                     1024 * c + so:1024 * c + so + sw],
                                stat[:, 8 * F_PER:8 * F_PER + 8],
                                xch[k][:, QPOS * c + so:QPOS * c + so + sw],
                                start=False, stop=False,
                                skip_group_check=True,
                                tile_position=(0, 32 * c),
                            )
                kind, r = prod[(j, k)]
                if kind == "v":
                    tensor.wait_ge(dve_sem, r + 1)
                    a = vring[r % RING_V]
                else:
                    tensor.wait_ge(actp_sem, r + 1)
                    a = sring[r % RING_S]
                is_last = (j, k) == units[-1]
                for c in range(NSTREAM):
                    for si, (so, sw) in enumerate(SUB):
                        fin = c == NSTREAM - 1 and si == len(SUB) - 1
                        mm = tensor.matmul(
                            psum[32 * c:32 * c + F_PER,
                                 1024 * c + so:1024 * c + so + sw],
                            stat[:, 8 * j:8 * j + 8],
                            a[:, QPOS * c + so:QPOS * c + so + sw],
                            start=False, stop=is_last and fin,
                            skip_group_check=True,
                            tile_position=(0, 32 * c),
                        )
                        if fin:
                            mm.then_inc(
                                pe_v_sem if kind == "v" else pe_s_sem, 1)
            # all matmuls retired in pc order once the last one is done
            tensor.wait_ge(pe_v_sem, NV)
            tensor.wait_ge(pe_s_sem, NS)
            tensor.nop().then_inc(pe_all_sem, 1)

    return nc


def _prep_inputs(x: np.ndarray, W: np.ndarray):
    x = np.asarray(x, dtype=np.float32)
    W = np.asarray(W, dtype=np.float32)
    # Host im2col in (kh, kw, c) d-order -> zero-padded (5, 128, POS) bf16
    xp = np.zeros((C, N, H + 2, W_SP + 2), np.float32)
    xp[:, :, PAD:PAD + H, PAD:PAD + W_SP] = x.transpose(1, 0, 2, 3)
    xc = np.zeros((NCHUNK * 128, POS), np.float32)
    for b in range(KK * KK):
        kh, kw = divmod(b, KK)
        xc[64 * b:64 * (b + 1), :] = (
            xp[:, :, kh:kh + H, kw:kw + W_SP].reshape(C, POS))
    xpad = xc.reshape(NCHUNK, 128, POS).astype(ml_dtypes.bfloat16)
    # W_col in (kh, kw, c) d-order: (F, 576)
    Wp = W.transpose(0, 2, 3, 1).reshape(F, KK * KK * C)
    wtiles = []
    for i in range(N_CORES):
        wt = np.zeros((128, WCOLS + 1), np.float32)
        for k in range(NCHUNK):
            dd = min(128, D - 128 * k)
            blk = Wp[F_PER * i:F_PER * (i + 1), 128 * k:128 * k + dd].T
            wt[:dd, k * F_PER:(k + 1) * F_PER] = blk
        sw = Wp[F_PER * i:F_PER * (i + 1), :].sum(axis=1)
        for c in range(NSTREAM):
            wt[32 * c:32 * c + F_PER, WCOLS] = sw
        wtiles.append(wt)
    return xpad, wtiles


_CACHED_NC = None
LAST_RESULT = None  # BassKernelResults of the most recent run (for test.py)


def kernel(x: np.ndarray, W: np.ndarray, _trace: bool = False) -> np.ndarray:
    global _CACHED_NC, LAST_RESULT
    xpad, wtiles = _prep_inputs(x, W)
    if _CACHED_NC is None:
        _CACHED_NC = build_bass()
    nc = _CACHED_NC
    in_maps = [{"xcol": xpad, "wcols": wtiles[i]} for i in range(N_CORES)]
    res = run_bass_kernel_spmd(nc, in_maps, core_ids=list(range(N_CORES)),
                               trace=_trace)
    LAST_RESULT = res
    outs = [np.asarray(res.results[i]["out"], dtype=np.float32)
            .transpose(1, 0, 2).reshape(F_PER, POS)
            for i in range(N_CORES)]
    o = np.concatenate(outs, axis=0)                    # (64, 3136)
    o = (o.reshape(F, N, P).transpose(1, 0, 2)
          .reshape(N, F, H, W_SP).astype(np.float32))
    return o


# revision 33
# speedup vs baseline: 1.0678x; 1.0154x over previous
"""AdderNet 2D conv (L1-distance "convolution") on 8 TRN2 NeuronCores.

Reference computation:
    X_col = unfold(x, k=3, stride=1, pad=1)      # (N, D, P)  D=576, P=196
    out[n, f, p] = -sum_d |W_col[f, d] - X_col[n, d, p]|

Distribution: filter-parallel — core i computes filters f in [8i, 8i+8)
for the FULL batch (no collectives; host concatenates filter slices).
This makes the per-instruction free dim N*P = 3136, which amortizes
per-instruction overhead far better than batch-parallel (392).

Per-core algorithm (raw Bass; this walrus encodes only ONE inline
sync-wait per instruction, so Tile's auto-semaphores don't compile —
standalone wait_ge instructions are used instead):

  -sum_d |x-w|  =  -sum_d x  + sum_d w  + 2*sum_d min(x-w, 0)

  - Host im2col: d (patch dim, 576, (kh,kw,c)-ordered) on SBUF
    partitions, 5 zero-padded chunks of 128, shipped as dense
    (128, 3136) bf16 tiles (window DMAs would shatter into 28-byte
    descriptors and starve everything).
  - Per (filter, chunk) unit, ONE elementwise instruction:
      VectorE: tensor_scalar(op0=sub W[f,.], op1=min 0)  -> min(x-w, 0)
      ScalarE: activation(Relu, scale=-1, bias=W[f,.])   -> relu(w-x)
    (no encodable fused abs exists on the Vector engine in this ISA;
    the f's are split between the engines to balance them).
  - TensorE reduces over partitions into PSUM, 4-way column-tiled:
    filter pair (2c, 2c+1) lives on PSUM partitions {32c, 32c+1} and
    its matmuls run on array column-group c — four concurrent moving
    streams (tile_position=(0, 32c)).  Stationary column = +2 (vector
    units) / -2 (scalar units); an all(-1) 2-col stationary block
    accumulates -sum_d x once per (chunk, stream).  PSUM is one
    (128, 7*512) tensor: matmul moving slices of 512 columns map
    exactly onto banks.  A zero-fill prologue (start=True, zero
    moving) initializes every bank's has_written bits so all real
    matmuls can accumulate without ordering hazards across streams.
  - Evacuation: per stream, psum rows {32c, 32c+1} + bias sum_d W[f]
    (activation bias / tensor_scalar add) -> osb rows {32c, 32c+1};
    streams 0-1 on ScalarE, 2-3 on VectorE.  The out DMA gathers the
    8 scattered osb rows with a 2-level partition access pattern.

kernel(x, W) accepts the FULL inputs and returns the FULL output.
"""

import os

import numpy as np
import ml_dtypes

import concourse.bass as bass
from concourse import mybir
from concourse.bass_utils import run_bass_kernel_spmd

# Problem constants (hardcoded per harness rules)
N, C, H, W_SP = 16, 64, 14, 14
F = 64
KK = 3
PAD = 1
P = H * W_SP            # 196 output positions per image
POS = N * P             # 3136 total positions
D = C * KK * KK         # 576
N_CORES = 8
F_PER = F // N_CORES    # 8 filters per core
NCHUNK = 5              # ceil(576 / 128) d-chunks
NSTREAM = 4             # TensorE column-tiling streams
QPOS = POS // NSTREAM   # 784 positions per stream (its private quarter)
BANKC = 512             # psum bank capacity in f32
SUB = [(0, 512), (512, 272)]  # per-stream sub-slices (2 private banks)
RING_V = 8              # vector-produced tile ring
RING_S = 3              # scalar-produced tile ring

FP32 = mybir.dt.float32
BF16 = mybir.dt.bfloat16

# Filters handled by the Scalar engine (the rest go to VectorE).
ACT_F = tuple(
    int(t) for t in os.environ.get("ADDER_ACT_F", "3,7").split(",") if t != ""
)

WCOLS = NCHUNK * F_PER  # 40 W columns (col = k*8+j)


def build_bass():
    nc = bass.Bass()

    x_ext = nc.declare_dram_parameter("xcol", [NCHUNK, 128, POS], BF16,
                                      isOutput=False)
    # cols 0:40 = W columns (col = k*8 + j); col 40 row (32*(j//2)+j%2) =
    # sum_d W[f_j]
    w_ext = nc.declare_dram_parameter("wcols", [128, WCOLS + 1], FP32,
                                      isOutput=False)
    out_ext = nc.declare_dram_parameter("out", [NSTREAM, F_PER, QPOS],
                                        FP32, isOutput=True)

    # SBUF
    w_sb = nc.alloc_sbuf_tensor("w_sb", [128, WCOLS + 1], FP32)
    # stationary: per-filter 8-col blocks (col j = +/-2) at [8j:8j+8],
    # all(-1) block at [64:72] (x sums), zeros at [72:80] (prologue)
    stat = nc.alloc_sbuf_tensor("stat", [128, 8 * F_PER + 16], BF16)
    zmov = nc.alloc_sbuf_tensor("zmov", [128, BANKC], BF16)
    xch = [nc.alloc_sbuf_tensor(f"xc{k}", [128, POS], BF16)
           for k in range(NCHUNK)]
    vring = [nc.alloc_sbuf_tensor(f"vb{r}", [128, POS], BF16)
             for r in range(RING_V)]
    sring = [nc.alloc_sbuf_tensor(f"sb{r}", [128, POS], BF16)
             for r in range(RING_S)]
    osb = nc.alloc_sbuf_tensor("osb", [128, QPOS], FP32)

    # PSUM: 8 banks; stream c owns banks {2c, 2c+1} = cols
    # [1024c, 1024c+784) and computes ALL 8 filters (rows 32c..32c+8)
    # for its private position quarter [784c, 784c+784).  Streams never
    # share a bank: concurrent accumulating matmuls on a shared bank
    # corrupt it (observed on HW).
    psum = nc.alloc_psum_tensor("ps", [128, 8 * BANKC], FP32)

    units = [(j, k) for k in range(NCHUNK) for j in range(F_PER)]
    prod = {}   # (j, k) -> ("v"|"s", producer-local index)
    nv = ns = 0
    for (j, k) in units:
        if j in ACT_F:
            prod[(j, k)] = ("s", ns)
            ns += 1
        else:
            prod[(j, k)] = ("v", nv)
            nv += 1
    NV, NS = nv, ns

    with (
        nc.Block() as block,
        nc.semaphore("w_sem") as w_sem,
        nc.semaphore("x0_sem") as x0_sem,
        nc.semaphore("x1_sem") as x1_sem,
        nc.semaphore("x2_sem") as x2_sem,
        nc.semaphore("x3_sem") as x3_sem,
        nc.semaphore("x4_sem") as x4_sem,
        nc.semaphore("out_sem") as out_sem,
        nc.semaphore("init_sem") as init_sem,
        nc.semaphore("dve_sem") as dve_sem,
        nc.semaphore("actp_sem") as actp_sem,
        nc.semaphore("pe_v_sem") as pe_v_sem,
        nc.semaphore("pe_s_sem") as pe_s_sem,
        nc.semaphore("evac_sem") as evac_sem,
        nc.semaphore("evac2_sem") as evac2_sem,
        nc.semaphore("st0_sem") as st0_sem,
        nc.semaphore("st1_sem") as st1_sem,
        nc.semaphore("st2_sem") as st2_sem,
        nc.semaphore("st3_sem") as st3_sem,
    ):
        xsem = [x0_sem, x1_sem, x2_sem, x3_sem, x4_sem]
        stsem = [st0_sem, st1_sem, st2_sem, st3_sem]
        xthr = [64, 64] + [32] * (NCHUNK - 2)  # 4/4/2-way splits

        @block.sync
        def _(sync: bass.BassEngine):
            # input DMAs are descriptor-rate-bound; chunk 0 gates the
            # whole pipeline, so give it exclusive DMA bandwidth first,
            # then stream the rest (they complete well before use).
            sync.dma_start(out=w_sb[:], in_=w_ext[:]).then_inc(w_sem, 16)
            for q in range(4):
                sync.dma_start(
                    out=xch[0][32 * q:32 * (q + 1), :],
                    in_=x_ext[0, 32 * q:32 * (q + 1), :]).then_inc(
                        x0_sem, 16)
            sync.wait_ge(x0_sem, 64)
            for q in range(4):
                sync.dma_start(
                    out=xch[1][32 * q:32 * (q + 1), :],
                    in_=x_ext[1, 32 * q:32 * (q + 1), :]).then_inc(
                        x1_sem, 16)
            for k in range(2, NCHUNK):
                for q in range(2):
                    sync.dma_start(
                        out=xch[k][64 * q:64 * (q + 1), :],
                        in_=x_ext[k, 64 * q:64 * (q + 1), :]).then_inc(
                            xsem[k], 16)
            # final output store: out[f, 784c + p] = osb[32c + f, p];
            # one plain 2-D DMA per stream (a single 3-D gather AP gets
            # mangled by the DMA AP optimizer); each goes as soon as its
            # evacuating engine finishes that stream
            for i, c in enumerate((0, 2, 1, 3)):
                sync.wait_ge(evac_sem if c < 2 else evac2_sem, 1 + (c % 2))
                sync.dma_start(
                    out=out_ext[c],
                    in_=osb[32 * c:32 * c + F_PER, :],
                ).then_inc(out_sem, 16)
            sync.wait_ge(out_sem, 16 * NSTREAM)

        @block.vector
        def _(vector: bass.BassEngine):
            # stationary: block j (cols 8j..8j+8): col j = +/-2;
            # x-sum block (cols 64:72) = -1; prologue cols 72:80 = 0
            vector.memset(stat[:], 0.0)
            for j in range(F_PER):
                val = -2.0 if j in ACT_F else 2.0
                vector.memset(stat[:, 8 * j + j:8 * j + j + 1], val)
            vector.memset(stat[:, 8 * F_PER:8 * F_PER + 8], -1.0)
            last = vector.memset(zmov[:], 0.0)
            last.then_inc(init_sem, 1)
            vector.wait_ge(w_sem, 16)
            seen = set()
            for (j, k) in units:
                kind, r = prod[(j, k)]
                if kind != "v":
                    continue
                if k not in seen:
                    seen.add(k)
                    vector.wait_ge(xsem[k], xthr[k])
                if r >= RING_V:
                    vector.wait_ge(pe_v_sem, r - RING_V + 1)
                col = k * F_PER + j
                vector.tensor_scalar(
                    out=vring[r % RING_V][:], in0=xch[k][:],
                    scalar1=w_sb[:, col:col + 1], scalar2=0.0,
                    op0=mybir.AluOpType.subtract,
                    op1=mybir.AluOpType.min,
                ).then_inc(dve_sem, 1)
            # evacuate streams 2..3 (psum rows 32c..32c+8 + sum_d W)
            for c in range(2, NSTREAM):
                vector.wait_ge(stsem[c], 1)
                vector.tensor_scalar(
                    out=osb[32 * c:32 * c + F_PER, :],
                    in0=psum[32 * c:32 * c + F_PER,
                             1024 * c:1024 * c + QPOS],
                    scalar1=w_sb[32 * c:32 * c + F_PER, WCOLS:WCOLS + 1],
                    scalar2=None,
                    op0=mybir.AluOpType.add,
                ).then_inc(evac2_sem, 1)

        @block.scalar
        def _(scalar: bass.BassEngine):
            # touch the Relu table first so the one-time ACT table load
            # overlaps the input DMAs instead of the first real unit
            scalar.activation(osb[0:1, 0:1], zmov[0:1, 0:1],
                              mybir.ActivationFunctionType.Relu,
                              bias=0.0, scale=1.0)
            scalar.wait_ge(w_sem, 16)
            seen = set()
            for (j, k) in units:
                kind, r = prod[(j, k)]
                if kind != "s":
                    continue
                if k not in seen:
                    seen.add(k)
                    scalar.wait_ge(xsem[k], xthr[k])
                if r >= RING_S:
                    scalar.wait_ge(pe_s_sem, r - RING_S + 1)
                col = k * F_PER + j
                scalar.activation(
                    sring[r % RING_S][:], xch[k][:],
                    mybir.ActivationFunctionType.Relu,
                    bias=w_sb[:, col:col + 1], scale=-1.0,
                ).then_inc(actp_sem, 1)
            # evacuate streams 0..1
            for c in range(0, 2):
                scalar.wait_ge(stsem[c], 1)
                scalar.activation(
                    osb[32 * c:32 * c + F_PER, :],
                    psum[32 * c:32 * c + F_PER, 1024 * c:1024 * c + QPOS],
                    mybir.ActivationFunctionType.Identity,
                    bias=w_sb[32 * c:32 * c + F_PER, WCOLS:WCOLS + 1],
                    scale=1.0,
                ).then_inc(evac_sem, 1)

        @block.tensor
        def _(tensor: bass.BassEngine):
            tensor.wait_ge(init_sem, 1)  # stat + zmov memsets done
            # prologue: zero each stream's psum rows in its private
            # banks; each bank's first matmul carries start=True
            for c in range(NSTREAM):
                for (so, sw) in SUB:
                    tensor.matmul(
                        psum[32 * c:32 * c + F_PER,
                             1024 * c + so:1024 * c + so + sw],
                        stat[:, 8 * F_PER + 8:8 * F_PER + 16],
                        zmov[:, 0:sw],
                        start=True, stop=False, skip_group_check=True,
                        tile_position=(0, 32 * c),
                    )
            kdone = set()
            for (j, k) in units:
                if k not in kdone:
                    kdone.add(k)
                    tensor.wait_ge(xsem[k], xthr[k])
                    # -sum_d x for chunk k (all filters) on every stream
                    for c in range(NSTREAM):
                        for (so, sw) in SUB:
                            tensor.matmul(
                                psum[32 * c:32 * c + F_PER,
                                     1024 * c + so:1024 * c + so + sw],
                                stat[:, 8 * F_PER:8 * F_PER + 8],
                                xch[k][:, QPOS * c + so:QPOS * c + so + sw],
                                start=False, stop=False,
                                skip_group_check=True,
                                tile_position=(0, 32 * c),
                            )
                kind, r = prod[(j, k)]
                if kind == "v":
                    tensor.wait_ge(dve_sem, r + 1)
                    a = vring[r % RING_V]
                else:
                    tensor.wait_ge(actp_sem, r + 1)
                    a = sring[r % RING_S]
                is_last = (j, k) == units[-1]
                for c in range(NSTREAM):
                    for si, (so, sw) in enumerate(SUB):
                        fin = c == NSTREAM - 1 and si == len(SUB) - 1
                        mm = tensor.matmul(
                            psum[32 * c:32 * c + F_PER,
                                 1024 * c + so:1024 * c + so + sw],
                            stat[:, 8 * j:8 * j + 8],
                            a[:, QPOS * c + so:QPOS * c + so + sw],
                            start=False, stop=is_last and fin,
                            skip_group_check=True,
                            tile_position=(0, 32 * c),
                        )
                        if is_last and si == len(SUB) - 1:
                            # stream c fully accumulated (pc order)
                            mm.then_inc(stsem[c], 1)
                        elif fin:
                            mm.then_inc(
                                pe_v_sem if kind == "v" else pe_s_sem, 1)

    return nc


def _prep_inputs(x: np.ndarray, W: np.ndarray):
    x = np.asarray(x, dtype=np.float32)
    W = np.asarray(W, dtype=np.float32)
    # Host im2col in (kh, kw, c) d-order -> zero-padded (5, 128, POS) bf16
    xp = np.zeros((C, N, H + 2, W_SP + 2), np.float32)
    xp[:, :, PAD:PAD + H, PAD:PAD + W_SP] = x.transpose(1, 0, 2, 3)
    xc = np.zeros((NCHUNK * 128, POS), np.float32)
    for b in range(KK * KK):
        kh, kw = divmod(b, KK)
        xc[64 * b:64 * (b + 1), :] = (
            xp[:, :, kh:kh + H, kw:kw + W_SP].reshape(C, POS))
    xpad = xc.reshape(NCHUNK, 128, POS).astype(ml_dtypes.bfloat16)
    # W_col in (kh, kw, c) d-order: (F, 576)
    Wp = W.transpose(0, 2, 3, 1).reshape(F, KK * KK * C)
    wtiles = []
    for i in range(N_CORES):
        wt = np.zeros((128, WCOLS + 1), np.float32)
        for k in range(NCHUNK):
            dd = min(128, D - 128 * k)
            blk = Wp[F_PER * i:F_PER * (i + 1), 128 * k:128 * k + dd].T
            wt[:dd, k * F_PER:(k + 1) * F_PER] = blk
        sw = Wp[F_PER * i:F_PER * (i + 1), :].sum(axis=1)
        for c in range(NSTREAM):
            wt[32 * c:32 * c + F_PER, WCOLS] = sw
        wtiles.append(wt)
    return xpad, wtiles


_CACHED_NC = None
LAST_RESULT = None  # BassKernelResults of the most recent run (for test.py)


def kernel(x: np.ndarray, W: np.ndarray, _trace: bool = False) -> np.ndarray:
    global _CACHED_NC, LAST_RESULT
    xpad, wtiles = _prep_inputs(x, W)
    if _CACHED_NC is None:
        _CACHED_NC = build_bass()
    nc = _CACHED_NC
    in_maps = [{"xcol": xpad, "wcols": wtiles[i]} for i in range(N_CORES)]
    res = run_bass_kernel_spmd(nc, in_maps, core_ids=list(range(N_CORES)),
                               trace=_trace)
    LAST_RESULT = res
    outs = [np.asarray(res.results[i]["out"], dtype=np.float32)
            .transpose(1, 0, 2).reshape(F_PER, POS)
            for i in range(N_CORES)]
    o = np.concatenate(outs, axis=0)                    # (64, 3136)
    o = (o.reshape(F, N, P).transpose(1, 0, 2)
          .reshape(N, F, H, W_SP).astype(np.float32))
    return o
